# revision 6
# baseline (speedup 1.0000x reference)
"""PointGRN (segment_reduce) Trainium2 Bass kernel.

Computation (per segment b, channel c over points feat [N, 64] f32):
    sumsq[b,c]  = sum_{n in seg b} feat[n,c]^2
    r[b,c]      = sqrt(sumsq[b,c])
    rn[b,c]     = r[b,c] / (mean_c r[b,:] + 1e-6)
    out[n,c]    = feat[n,c] * (1 + gamma[c]*rn[b,c]) + beta[c]

Sharding: data-parallel over segments — host reads `offset` and gives each
of the 8 cores one whole segment (padded with zero rows to a 128-row
multiple).  No device-side searchsorted and no collectives needed.

Device kernel (per core), DMA-bound (~360-410 GB/s/core HBM ceiling,
shared load+store):
    pass 1: stream [128 x k*64] f32 tiles; EVERY tile is kept resident in
            SBUF as fp16 (16 MB total — the whole shard stays on-chip, no
            pass-2 reload).  Loads are spread over all three DMA issue
            paths (SWDGE with f32->fp16 cast in-flight, plus the two HWDGE
            rings staging f32 through the store pool + DVE downcast).
            ACT squares into bf16; PE ones-matmul reduces partitions into
            4 PSUM banks (chunk j of each tile accumulates into bank j%4).
    combine: tiny [1,64] vector math (sqrt + Newton step, mean, scale),
            broadcast scale/beta to [128,128] via a K=1 matmul.
    pass 2: y = fp16_resident * s + beta into f32 staging tiles; store.
            Mult on DVE, +beta alternates DVE/GPSIMD, stores cycle the
            DMA issue paths.
    fp16 residency costs ~4e-4 relative error on the scaled term — far
    inside the 2e-2 gate; the sumsq path keeps bf16 squares with f32 PSUM
    accumulation.
"""

import numpy as np

import concourse.bacc as bacc
import concourse.bass as bass
import concourse.mybir as mybir
import concourse.tile as tile
from concourse.bass_utils import run_bass_kernel_spmd

EPS = 1e-06
N_CORES = 8
P = 128          # SBUF partitions
C = 64           # channels
K = 32           # row-groups per partition per full tile (default)
MM_N = 512       # matmul moving free-dim chunk (one PSUM bank)
NACC = 4         # PSUM accumulator banks for the sumsq reduction

_AFT = mybir.ActivationFunctionType
_ALU = mybir.AluOpType

_program_cache: dict[tuple, bass.Bass] = {}


def _tile_rows(r_pad, k_tile):
    """Split r_pad rows into full [128 x k_tile] tiles plus one ragged tail."""
    pchunks = r_pad // P
    nt_full = pchunks // k_tile
    k_tail = pchunks % k_tile
    ks = [k_tile] * nt_full + ([k_tail] if k_tail else [])
    return ks


def _build_program(
    r_pad: int,
    repeats: int = 1,
    k_tile: int = K,
    cast_every: int = 2,   # every Nth tile loads via SWDGE f32->fp16 cast; 0 = never
    store3: int = 0,       # 1: stores also cycle through SWDGE
    bufs_y: int = 4,
) -> bass.Bass:
    """One-core Bass program for a shard of r_pad rows (r_pad % 128 == 0).

    `repeats` re-runs the whole computation body that many times (timing
    only: the wall-clock slope over repeats isolates kernel time from the
    flat dispatch overhead of this axon environment).
    """
    from contextlib import ExitStack

    ks = _tile_rows(r_pad, k_tile)
    nt = len(ks)
    F = k_tile * C
    nc = bacc.Bacc()

    feat = nc.declare_dram_parameter("feat", [r_pad, C], mybir.dt.float32, isOutput=False)
    gamma = nc.declare_dram_parameter("gamma", [1, C], mybir.dt.float32, isOutput=False)
    beta = nc.declare_dram_parameter("beta", [1, C], mybir.dt.float32, isOutput=False)
    out = nc.declare_dram_parameter("out", [r_pad, C], mybir.dt.float32, isOutput=True)

    row0 = [0] * nt
    for t in range(1, nt):
        row0[t] = row0[t - 1] + P * ks[t - 1]

    def feat_view(t):
        r0 = row0[t]
        return feat[r0 : r0 + P * ks[t], :].rearrange("(p k) c -> p (k c)", k=ks[t])

    def out_view(t):
        r0 = row0[t]
        return out[r0 : r0 + P * ks[t], :].rearrange("(p k) c -> p (k c)", k=ks[t])

    def is_cast(t):
        return cast_every > 0 and t % cast_every == 0

    # PSUM bank bookkeeping: chunk j of tile t accumulates into bank j % NACC.
    # start on the bank's first write, stop on its last.
    first_write = {}
    last_write = {}
    for t in range(nt):
        for j in range((ks[t] * C + MM_N - 1) // MM_N):
            b = j % NACC
            if b not in first_write:
                first_write[b] = (t, j)
            last_write[b] = (t, j)

    with tile.TileContext(nc) as tc, ExitStack() as ctx:
        const = ctx.enter_context(tc.tile_pool(name="const", bufs=1))
        resp = ctx.enter_context(tc.tile_pool(name="resp", bufs=1))
        outp = ctx.enter_context(tc.tile_pool(name="outp", bufs=bufs_y))
        sqp = ctx.enter_context(tc.tile_pool(name="sqp", bufs=2))
        psum = ctx.enter_context(tc.tile_pool(name="psum", bufs=1, space="PSUM"))
        small = ctx.enter_context(tc.tile_pool(name="small", bufs=1))

        ones_col = const.tile([P, 1], mybir.dt.bfloat16, name="ones_col", tag="ones_col")
        nc.vector.memset(ones_col, 1.0)
        ones_row = const.tile([1, P], mybir.dt.float32, name="ones_row", tag="ones_row")
        nc.vector.memset(ones_row, 1.0)

        # prefetch gamma/beta off the combine critical path
        g_row = const.tile([1, C], mybir.dt.float32, name="g_row", tag="g_row")
        nc.sync.dma_start(out=g_row, in_=gamma[:])
        b_row = const.tile([1, C], mybir.dt.float32, name="b_row", tag="b_row")
        nc.sync.dma_start(out=b_row, in_=beta[:])

        # [P,1] ones for the pass-1 DVE downcast: tensor_tensor(mult) stays in
        # 1-port mode and never locks GPSIMD out of SBUF, unlike tensor_copy /
        # tensor_scalar which enter 2-port perf mode and starve SWDGE
        # descriptor generation (the concurrent cast-DMA loads).
        ones_p = const.tile([P, 1], mybir.dt.float32, name="ones_p", tag="ones_p")
        nc.vector.memset(ones_p, 1.0)

        def ones_bc(f_t):
            return bass.AP(
                tensor=ones_p.tensor, offset=ones_p.offset, ap=[ones_p.ap[0], [0, f_t]]
            )

        for _rep in range(repeats):
            # --- pass 1: sum of squares + fp16 residency ------------------
            acc = [
                psum.tile([1, MM_N], mybir.dt.float32, name=f"acc{b}", tag=f"acc{b}")
                for b in range(NACC)
            ]
            res_tiles = []
            n_hw = 0
            for t in range(nt):
                f_t = ks[t] * C
                xr = resp.tile([P, f_t], mybir.dt.float16, name="xr", tag=f"res{t}")
                res_tiles.append(xr)
                if is_cast(t):
                    # SWDGE casts f32->fp16 in the DMA datapath
                    nc.gpsimd.dma_start(out=xr, in_=feat_view(t))
                    sq_src = xr
                else:
                    # stage f32 through the (pass-1-idle) store pool
                    x = outp.tile([P, F], mybir.dt.float32, name="y", tag="y")[:, :f_t]
                    eng = nc.sync if n_hw % 2 == 0 else nc.scalar
                    n_hw += 1
                    eng.dma_start(out=x, in_=feat_view(t))
                    nc.vector.tensor_tensor(xr, x, ones_bc(f_t), _ALU.mult)
                    sq_src = x
                sq = sqp.tile([P, F], mybir.dt.bfloat16, name="sq", tag="sq")
                nc.scalar.activation(sq[:, :f_t], sq_src, _AFT.Square)
                for j in range((f_t + MM_N - 1) // MM_N):
                    w = min(MM_N, f_t - j * MM_N)
                    b = j % NACC
                    nc.tensor.matmul(
                        acc[b][:, :w],
                        lhsT=ones_col[:, :],
                        rhs=sq[:, j * MM_N : j * MM_N + w],
                        start=(first_write[b] == (t, j)),
                        stop=(last_write[b] == (t, j)),
                    )

            # --- combine: [1,64] vector math ------------------------------
            red = small.tile([1, NACC, C], mybir.dt.float32, name="red", tag="red")
            for b in range(NACC):
                nc.vector.tensor_reduce(
                    out=red[:, b, :],
                    in_=acc[b][:, :].rearrange("p (k c) -> p c k", c=C),
                    axis=mybir.AxisListType.X,
                    op=_ALU.add,
                )
            sumsq = small.tile([1, C], mybir.dt.float32, name="sumsq", tag="sumsq")
            nc.vector.tensor_reduce(
                out=sumsq,
                in_=red[:, :, :].rearrange("p k c -> p c k"),
                axis=mybir.AxisListType.X,
                op=_ALU.add,
            )

            # r2 = 2*sqrt(sumsq) via ACT sqrt + one Newton step (ACT sqrt is
            # low precision; Newton with the accurate DVE reciprocal fixes it)
            r0 = small.tile([1, C], mybir.dt.float32, name="r0", tag="r0")
            nc.scalar.activation(r0, sumsq, _AFT.Sqrt)
            rm = small.tile([1, C], mybir.dt.float32, name="rm", tag="rm")
            nc.vector.tensor_scalar_max(rm, r0, 1e-30)
            rinv = small.tile([1, C], mybir.dt.float32, name="rinv", tag="rinv")
            nc.vector.reciprocal(rinv, rm)
            t1 = small.tile([1, C], mybir.dt.float32, name="t1", tag="t1")
            nc.vector.tensor_mul(t1, sumsq, rinv)
            r2 = small.tile([1, C], mybir.dt.float32, name="r2", tag="r2")
            nc.vector.tensor_add(r2, r0, t1)

            # mean + eps:  me = sum(r2)/128 + EPS   (r2 = 2r -> mean = sum/128)
            msum = small.tile([1, 1], mybir.dt.float32, name="msum", tag="msum")
            nc.vector.tensor_reduce(out=msum, in_=r2, axis=mybir.AxisListType.X, op=_ALU.add)
            eps_t = small.tile([1, 1], mybir.dt.float32, name="eps_t", tag="eps_t")
            nc.vector.memset(eps_t, EPS)
            me = small.tile([1, 1], mybir.dt.float32, name="me", tag="me")
            nc.scalar.activation(me, msum, _AFT.Identity, bias=eps_t[:, :], scale=1.0 / (2 * C))
            minv = small.tile([1, 1], mybir.dt.float32, name="minv", tag="minv")
            nc.vector.reciprocal(minv, me)
            mh = small.tile([1, 1], mybir.dt.float32, name="mh", tag="mh")
            nc.vector.tensor_scalar_mul(mh, minv, 0.5)

            # s = 1 + gamma * (r2 * 0.5 * minv); pack [s | beta] in one row
            t2 = small.tile([1, C], mybir.dt.float32, name="t2", tag="t2")
            nc.vector.tensor_mul(t2, r2, g_row)
            sb_cat = small.tile([1, 2 * C], mybir.dt.float32, name="sb_cat", tag="sb_cat")
            nc.vector.tensor_scalar(
                sb_cat[:, 0:C], t2, scalar1=mh[:, :], scalar2=1.0, op0=_ALU.mult, op1=_ALU.add
            )
            nc.vector.tensor_copy(out=sb_cat[:, C : 2 * C], in_=b_row)

            # broadcast [1,128] -> [128,128]: cols 0-63 = s, 64-127 = beta
            bc_ps = psum.tile([P, 2 * C], mybir.dt.float32, name="bc_ps", tag="bc_ps")
            nc.tensor.matmul(bc_ps[:, :], lhsT=ones_row[:, :], rhs=sb_cat[:, :], start=True, stop=True)
            sb_bc = small.tile([P, 2 * C], mybir.dt.float32, name="sb_bc", tag="sb_bc")
            nc.scalar.copy(sb_bc, bc_ps)
            s_bc = sb_bc[:, 0:C]
            b_bc = sb_bc[:, C : 2 * C]

            def bcast_ap(col_slice, kk):
                return bass.AP(
                    tensor=col_slice.tensor,
                    offset=col_slice.offset,
                    ap=[col_slice.ap[0], [0, kk], col_slice.ap[1]],
                )

            # --- pass 2: y = xr*s + beta -> f32 staging, store ------------
            for t in range(nt):
                f_t = ks[t] * C
                kk = ks[t]
                y = outp.tile([P, F], mybir.dt.float32, name="y", tag="y")[:, :f_t]
                x3 = res_tiles[t].rearrange("p (k c) -> p k c", c=C)
                y3 = y.rearrange("p (k c) -> p k c", c=C)
                nc.vector.tensor_tensor(y3, x3, bcast_ap(s_bc, kk), _ALU.mult)
                # alternate the +beta between DVE and Pool so neither engine
                # becomes the pass-2 critical path
                eng_add = nc.vector if t % 2 == 0 else nc.gpsimd
                eng_add.tensor_tensor(y3, y3, bcast_ap(b_bc, kk), _ALU.add)
                if store3:
                    eng_st = (nc.scalar, nc.sync, nc.gpsimd)[t % 3]
                else:
                    eng_st = nc.scalar if t % 2 == 0 else nc.sync
                eng_st.dma_start(out=out_view(t), in_=y)

    nc.finalize()
    return nc


def kernel(feat: np.ndarray, offset: np.ndarray, gamma: np.ndarray, beta: np.ndarray) -> np.ndarray:
    feat = np.ascontiguousarray(np.asarray(feat, dtype=np.float32))
    offset = np.asarray(offset)
    gamma = np.ascontiguousarray(np.asarray(gamma, dtype=np.float32)).reshape(1, C)
    beta = np.ascontiguousarray(np.asarray(beta, dtype=np.float32)).reshape(1, C)

    n = feat.shape[0]
    b = offset.shape[0]
    assert b <= N_CORES, f"need <= {N_CORES} segments, got {b}"

    ends = offset.astype(np.int64)
    starts = np.concatenate([[0], ends[:-1]])
    seg_rows = (ends - starts).astype(np.int64)

    r_max = int(seg_rows.max()) if b else P
    r_pad = max(P, ((r_max + P - 1) // P) * P)

    key = (r_pad,)
    nc = _program_cache.get(key)
    if nc is None:
        nc = _build_program(r_pad)
        _program_cache[key] = nc

    in_maps = []
    for i in range(N_CORES):
        shard = np.zeros((r_pad, C), dtype=np.float32)
        if i < b and seg_rows[i] > 0:
            shard[: seg_rows[i]] = feat[starts[i] : ends[i]]
        in_maps.append({"feat": shard, "gamma": gamma, "beta": beta})

    results = run_bass_kernel_spmd(nc, in_maps, core_ids=list(range(N_CORES))).results

    out_full = np.empty((n, C), dtype=np.float32)
    for i in range(b):
        if seg_rows[i] > 0:
            out_full[starts[i] : ends[i]] = results[i]["out"][: seg_rows[i]]

    # Rows past offset[-1] (possible with general sorted offsets): the
    # reference's searchsorted yields index b there, which jax clamps to
    # b-1 on gather — those rows are scaled by the last segment's rn but
    # excluded from its sumsq.  Replicate on host.
    tail0 = int(ends[-1]) if b else 0
    if tail0 < n:
        last0, last1 = int(starts[-1]), int(ends[-1])
        sumsq = (feat[last0:last1].astype(np.float64) ** 2).sum(axis=0)
        r = np.sqrt(sumsq)
        rn = (r / (r.mean() + EPS)).astype(np.float32)
        ft = feat[tail0:]
        out_full[tail0:] = ft + gamma * (ft * rn[None, :]) + beta
    return out_full
